# revision 16
# baseline (speedup 1.0000x reference)
"""Causal squeeze-excite 1d on 8 TRN2 NeuronCores.

Reference computation (per batch b):
    y = causal_ema(x)                      # y[t] = (1-a) y[t-1] + a x[t], y[0] = x[0]
    h = relu(w1 @ y[:, t] + b1)            # (32,)  per time step
    g = sigmoid(w2 @ h + b2)               # (512,) per time step
    out[:, t] = x[:, t] * g

Sharding: data-parallel over batch. Core i gets x[2i:2i+2]; the tiny MLP
weights are replicated.

Per-core kernel:
  - EMA via DVE tensor_tensor_scan on [128, Tc] tiles (state = d*state + x_t).
    We scan the rescaled sequence u = y/a (u_0 = cw*x_0, u_t = d u_{t-1} + x_t)
    so no a*x pre-scale pass is needed; the 'a' is folded into w1 on the host.
  - MLP1/MLP2: float32r matmuls (1 cyc/row vs fp32's 4) contracting C across
    4 partition blocks into PSUM; producer tiles (u, h, weights) are typed
    float32r so the walrus verifier accepts them as rounded inputs.
  - Relu/Sigmoid (+bias) on the scalar engine straight out of PSUM.
  - Gate multiply in-place into the x tile on DVE, then DMA out.
"""

import numpy as np
from contextlib import ExitStack

import concourse.bass as bass
import concourse.bacc as bacc
import concourse.tile as tile
import concourse.mybir as mybir
from concourse.bass_utils import run_bass_kernel_spmd

F32 = mybir.dt.float32
F32R = mybir.dt.float32r

N_CORES = 8
B, C, T = 16, 512, 4096
CSQ = 32          # squeeze dim
P = 128           # SBUF partitions


def build_nc(B_loc, cw, C_=C, T_=T, Tc=1024, TS=512):
    """Build the per-core Bass program. Shapes are compile-time constants."""
    d = 1.0 - 1.0 / cw
    NCB = C_ // P      # channel blocks
    NTH = T_ // Tc     # time chunks
    NTS = Tc // TS     # matmul sub-blocks per chunk

    nc = bacc.Bacc(trn_type="TRN2")
    x = nc.declare_dram_parameter("x", [B_loc, C_, T_], F32, isOutput=False)
    w1sT = nc.declare_dram_parameter("w1sT", [C_, CSQ], F32R, isOutput=False)
    b1 = nc.declare_dram_parameter("b1", [CSQ, 1], F32, isOutput=False)
    w2T = nc.declare_dram_parameter("w2T", [CSQ, C_], F32R, isOutput=False)
    b2 = nc.declare_dram_parameter("b2", [P, NCB], F32, isOutput=False)
    out = nc.declare_dram_parameter("out", [B_loc, C_, T_], F32, isOutput=True)

    with ExitStack() as ctx:
        tc = ctx.enter_context(tile.TileContext(nc))
        const = ctx.enter_context(tc.tile_pool(name="const", bufs=1))
        # One coalesced x tile per (b, th) chunk: [P, NCB*Tc] = 4 MiB, so a
        # single max-efficiency DMA covers the whole chunk. 3 bufs = 3 chunks
        # in flight keeps the DMA rings streaming during compute.
        xpool = ctx.enter_context(tc.tile_pool(name="xp", bufs=4))
        upool = ctx.enter_context(tc.tile_pool(name="up", bufs=2 * NCB))
        hpool = ctx.enter_context(tc.tile_pool(name="hp", bufs=4))
        gpool = ctx.enter_context(tc.tile_pool(name="gp", bufs=6))
        cpool = ctx.enter_context(tc.tile_pool(name="cp", bufs=2 * NCB))
        php = ctx.enter_context(tc.tile_pool(name="php", bufs=4, space="PSUM"))
        pgp = ctx.enter_context(tc.tile_pool(name="pgp", bufs=4, space="PSUM"))

        dconst = const.tile([P, Tc], F32, tag="dconst")
        nc.vector.memset(dconst[:], d)
        w1_t = []
        for cb in range(NCB):
            wt = const.tile([P, CSQ], F32R, tag=f"w1_{cb}")
            nc.sync.dma_start(wt[:], w1sT[cb * P:(cb + 1) * P, :])
            w1_t.append(wt)
        b1_t = const.tile([CSQ, 1], F32, tag="b1")
        nc.sync.dma_start(b1_t[:], b1[:])
        w2_t = const.tile([CSQ, C_], F32R, tag="w2")
        nc.sync.dma_start(w2_t[:], w2T[:])
        b2_t = const.tile([P, NCB], F32, tag="b2")
        nc.sync.dma_start(b2_t[:], b2[:])

        # DRAM views with channel blocks folded into the free dim:
        # [B, P, NCB, T] so one DMA moves a whole (b, th) chunk.
        xv = x.rearrange("b (cb p) t -> b p cb t", p=P)
        ov = out.rearrange("b (cb p) t -> b p cb t", p=P)

        # Interleave the independent batch streams: the scan chain serializes
        # along th within one (b, cb), so alternating b gives the pipeline a
        # second independent stream and hides the chain latency.
        carries = {b: [None] * NCB for b in range(B_loc)}
        for th in range(NTH):
            for b in range(B_loc):
                carry = carries[b]
                xt = xpool.tile([P, NCB * Tc], F32, tag="x")
                xt3 = xt[:].rearrange("p (cb t) -> p cb t", cb=NCB)
                nc.sync.dma_start(
                    xt3, xv[b, :, :, th * Tc:(th + 1) * Tc])
                uts = []
                for cb in range(NCB):
                    xs = xt[:, cb * Tc:(cb + 1) * Tc]
                    ut = upool.tile([P, Tc], F32R, tag="u")
                    if th == 0:
                        init = cpool.tile([P, 1], F32, tag="c")
                        nc.scalar.mul(init[:], xs[:, 0:1], float(cw))
                        init_ap = init[:]
                    else:
                        init_ap = carry[cb][:]
                    # Scans must run on DVE: walrus codegen rejects
                    # tensor_tensor_scan on Pool (GPSIMD).
                    nc.vector.tensor_tensor_scan(
                        ut[:], dconst[:], xs, init_ap,
                        mybir.AluOpType.mult, mybir.AluOpType.add)
                    if th + 1 < NTH:
                        # Stash the carry so the u tile slot can recycle
                        # without extending its lifetime into the next chunk.
                        cnext = cpool.tile([P, 1], F32, tag="c")
                        nc.scalar.copy(cnext[:], ut[:, Tc - 1:Tc])
                        carry[cb] = cnext
                    uts.append(ut)
                # Sigmoid outputs stage into one G tile per channel block so
                # the gate multiply runs as a single [P, Tc] op per cb —
                # fewer, larger DVE/GPSIMD instructions amortize the fixed
                # per-op cost.
                gts = [gpool.tile([P, Tc], F32, tag="g", name=f"g{cb}")
                       for cb in range(NCB)]
                for ts in range(NTS):
                    ph = php.tile([CSQ, TS], F32, tag="ph")
                    for cb in range(NCB):
                        nc.tensor.matmul(
                            ph[:], w1_t[cb][:],
                            uts[cb][:, ts * TS:(ts + 1) * TS],
                            start=(cb == 0), stop=(cb == NCB - 1))
                    ht = hpool.tile([CSQ, TS], F32R, tag="h")
                    nc.scalar.activation(
                        ht[:], ph[:], mybir.ActivationFunctionType.Relu,
                        bias=b1_t[:])
                    for cb in range(NCB):
                        pg = pgp.tile([P, TS], F32, tag="pg")
                        nc.tensor.matmul(
                            pg[:], w2_t[:, cb * P:(cb + 1) * P],
                            ht[:], start=True, stop=True)
                        nc.scalar.activation(
                            gts[cb][:, ts * TS:(ts + 1) * TS], pg[:],
                            mybir.ActivationFunctionType.Sigmoid,
                            bias=b2_t[:, cb:cb + 1])
                for cb in range(NCB):
                    # Gate multiply in place into the x tile. DVE does
                    # 1 cyc/col but carries the scans (2 cyc/col); GPSIMD is
                    # ~2x slower per col but otherwise idle. 1 DVE / 3 GPSIMD
                    # per chunk balances both near ~77us.
                    sl = slice(cb * Tc, (cb + 1) * Tc)
                    mul_eng = nc.vector if cb == (th + b) % NCB else nc.gpsimd
                    mul_eng.tensor_mul(xt[:, sl], xt[:, sl], gts[cb][:])
                nc.scalar.dma_start(
                    ov[b, :, :, th * Tc:(th + 1) * Tc], xt3)
    nc.compile()
    return nc


def make_in_maps(x, w1, b1, w2, b2, cw, n_cores=N_CORES):
    """Host-side shard + weight prep. Returns per-core input maps."""
    a = 1.0 / cw
    w1sT = np.ascontiguousarray((w1 * a).T, dtype=np.float32)      # [C, CSQ]
    b1c = np.ascontiguousarray(b1.reshape(-1, 1), dtype=np.float32)
    w2T = np.ascontiguousarray(w2.T, dtype=np.float32)             # [CSQ, C]
    ncb = w2.shape[0] // P
    b2c = np.ascontiguousarray(b2.reshape(ncb, P).T, dtype=np.float32)  # [P, NCB]
    b_loc = x.shape[0] // n_cores
    return [
        {
            "x": np.ascontiguousarray(x[i * b_loc:(i + 1) * b_loc], dtype=np.float32),
            "w1sT": w1sT, "b1": b1c, "w2T": w2T, "b2": b2c,
        }
        for i in range(n_cores)
    ]


_NC_CACHE = {}


def kernel(x, w1, b1, w2, b2, context_window):
    cw = int(context_window)
    x = np.asarray(x)
    key = (cw, x.shape)
    if key not in _NC_CACHE:
        _NC_CACHE[key] = build_nc(x.shape[0] // N_CORES, cw)
    nc = _NC_CACHE[key]
    in_maps = make_in_maps(
        np.asarray(x), np.asarray(w1), np.asarray(b1),
        np.asarray(w2), np.asarray(b2), cw)
    res = run_bass_kernel_spmd(nc, in_maps, core_ids=list(range(N_CORES)))
    return np.concatenate([r["out"] for r in res.results], axis=0)


# revision 18
# speedup vs baseline: 1.3045x; 1.3045x over previous
"""Causal squeeze-excite 1d on 8 TRN2 NeuronCores.

Reference computation (per batch b):
    y = causal_ema(x)                      # y[t] = (1-a) y[t-1] + a x[t], y[0] = x[0]
    h = relu(w1 @ y[:, t] + b1)            # (32,)  per time step
    g = sigmoid(w2 @ h + b2)               # (512,) per time step
    out[:, t] = x[:, t] * g

Sharding: data-parallel over batch. Core i gets x[2i:2i+2]; the tiny MLP
weights are replicated.

Key algebraic identity: the EMA is linear with channel-independent
coefficients, so it commutes with the channel projection:
    w1 @ ema(x) == ema(w1 @ x).
The kernel therefore projects first (p = (a*w1) @ x on the TensorEngine,
contracting C=512) and scans p — a [32, T] sequence — instead of the
[512, T] input. That cuts the DVE scan (the only engine that can run
tensor_tensor_scan, at 2 cyc/col) by 4x, from ~78us to ~17us per core.

Per-core pipeline, chunked along T with all streams double-buffered:
  - one coalesced DMA loads a [128, 4*Tc] x chunk (all 4 channel blocks);
  - mm1 (float32r) accumulates the 4 channel blocks into PSUM p tiles;
  - DVE tensor_tensor_scan consumes p straight out of PSUM
    (u_t = (1-a) u_{t-1} + p_t, rescaled so y = a*u is folded into w1);
  - relu(+b1) -> h (float32r), mm2 (float32r) -> PSUM, sigmoid(+b2) -> G;
  - gate multiply x*G into a separate output tile (DVE/GPSIMD split);
  - one coalesced DMA stores the [128, 4*Tc] output chunk.
float32r matmuls run 4x faster than fp32 on the PE; producer tiles are
typed float32r so the walrus verifier accepts them as rounded inputs.
"""

import numpy as np
from contextlib import ExitStack

import concourse.bass as bass
import concourse.bacc as bacc
import concourse.tile as tile
import concourse.mybir as mybir
from concourse.bass_utils import run_bass_kernel_spmd

F32 = mybir.dt.float32
F32R = mybir.dt.float32r

N_CORES = 8
B, C, T = 16, 512, 4096
CSQ = 32          # squeeze dim
P = 128           # SBUF partitions


def build_nc(B_loc, cw, C_=C, T_=T, Tc=1024, TS=512):
    """Build the per-core Bass program. Shapes are compile-time constants."""
    d = 1.0 - 1.0 / cw
    NCB = C_ // P      # channel blocks
    NTH = T_ // Tc     # time chunks
    NTS = Tc // TS     # matmul sub-blocks per chunk

    nc = bacc.Bacc(trn_type="TRN2")
    x = nc.declare_dram_parameter("x", [B_loc, C_, T_], F32R, isOutput=False)
    w1sT = nc.declare_dram_parameter("w1sT", [C_, CSQ], F32R, isOutput=False)
    b1 = nc.declare_dram_parameter("b1", [CSQ, 1], F32, isOutput=False)
    w2T = nc.declare_dram_parameter("w2T", [CSQ, C_], F32R, isOutput=False)
    b2 = nc.declare_dram_parameter("b2", [P, NCB], F32, isOutput=False)
    out = nc.declare_dram_parameter("out", [B_loc, C_, T_], F32, isOutput=True)

    with ExitStack() as ctx:
        tc = ctx.enter_context(tile.TileContext(nc))
        const = ctx.enter_context(tc.tile_pool(name="const", bufs=1))
        # One coalesced x tile per (b, th) chunk: [P, NCB*Tc], so a single
        # max-efficiency DMA covers the whole chunk; 4 bufs keep the DMA
        # rings streaming while compute runs.
        xpool = ctx.enter_context(tc.tile_pool(name="xp", bufs=4))
        opool = ctx.enter_context(tc.tile_pool(name="op", bufs=3))
        upool = ctx.enter_context(tc.tile_pool(name="up", bufs=6))
        hpool = ctx.enter_context(tc.tile_pool(name="hp", bufs=6))
        gpool = ctx.enter_context(tc.tile_pool(name="gp", bufs=6))
        cpool = ctx.enter_context(tc.tile_pool(name="cp", bufs=4))
        php = ctx.enter_context(tc.tile_pool(name="php", bufs=4, space="PSUM"))
        pgp = ctx.enter_context(tc.tile_pool(name="pgp", bufs=4, space="PSUM"))

        dconst = const.tile([CSQ, TS], F32, tag="dconst")
        nc.vector.memset(dconst[:], d)
        w1_t = []
        for cb in range(NCB):
            wt = const.tile([P, CSQ], F32R, tag=f"w1_{cb}")
            nc.sync.dma_start(wt[:], w1sT[cb * P:(cb + 1) * P, :])
            w1_t.append(wt)
        b1_t = const.tile([CSQ, 1], F32, tag="b1")
        nc.sync.dma_start(b1_t[:], b1[:])
        w2_t = const.tile([CSQ, C_], F32R, tag="w2")
        nc.sync.dma_start(w2_t[:], w2T[:])
        b2_t = const.tile([P, NCB], F32, tag="b2")
        nc.sync.dma_start(b2_t[:], b2[:])

        # DRAM views with channel blocks folded into the free dim:
        # [B, P, NCB, T] so one DMA moves a whole (b, th) chunk.
        xv = x.rearrange("b (cb p) t -> b p cb t", p=P)
        ov = out.rearrange("b (cb p) t -> b p cb t", p=P)

        # Interleave the independent batch streams: the scan chain serializes
        # along th within one b, so alternating b keeps a second independent
        # stream in flight.
        carries = {b: None for b in range(B_loc)}
        mul_idx = 0
        for th in range(NTH):
            for b in range(B_loc):
                xt = xpool.tile([P, NCB * Tc], F32R, tag="x")
                xt3 = xt[:].rearrange("p (cb t) -> p cb t", cb=NCB)
                nc.sync.dma_start(
                    xt3, xv[b, :, :, th * Tc:(th + 1) * Tc])
                ot = opool.tile([P, NCB * Tc], F32, tag="o")
                ot3 = ot[:].rearrange("p (cb t) -> p cb t", cb=NCB)
                gts = [gpool.tile([P, Tc], F32, tag="g", name=f"g{cb}")
                       for cb in range(NCB)]
                for ts in range(NTS):
                    # p = (a*w1) @ x, contracting C across the 4 channel
                    # blocks into one PSUM tile.
                    ph = php.tile([CSQ, TS], F32, tag="ph")
                    for cb in range(NCB):
                        nc.tensor.matmul(
                            ph[:], w1_t[cb][:],
                            xt[:, cb * Tc + ts * TS:cb * Tc + (ts + 1) * TS],
                            start=(cb == 0), stop=(cb == NCB - 1))
                    # EMA scan straight out of PSUM: u_t = d*u_{t-1} + p_t.
                    ut = upool.tile([CSQ, TS], F32R, tag="u")
                    if th == 0 and ts == 0:
                        init = cpool.tile([CSQ, 1], F32, tag="c")
                        nc.scalar.mul(init[:], ph[:, 0:1], float(cw))
                        init_ap = init[:]
                    else:
                        # Chain straight off the previous u tile: DVE-to-DVE
                        # stays in engine order, no cross-engine hop.
                        init_ap = carries[b][:, TS - 1:TS]
                    nc.vector.tensor_tensor_scan(
                        ut[:], dconst[:], ph[:], init_ap,
                        mybir.AluOpType.mult, mybir.AluOpType.add)
                    carries[b] = ut
                    ht = hpool.tile([CSQ, TS], F32R, tag="h")
                    nc.scalar.activation(
                        ht[:], ut[:], mybir.ActivationFunctionType.Relu,
                        bias=b1_t[:])
                    for cb in range(NCB):
                        pg = pgp.tile([P, TS], F32, tag="pg")
                        nc.tensor.matmul(
                            pg[:], w2_t[:, cb * P:(cb + 1) * P],
                            ht[:], start=True, stop=True)
                        nc.scalar.activation(
                            gts[cb][:, ts * TS:(ts + 1) * TS], pg[:],
                            mybir.ActivationFunctionType.Sigmoid,
                            bias=b2_t[:, cb:cb + 1])
                for cb in range(NCB):
                    # Gate multiply into the output tile. DVE does ~1.4us per
                    # [P, Tc] op, GPSIMD ~3us but is otherwise idle; an
                    # 18/14 split balances both near ~43us.
                    sl = slice(cb * Tc, (cb + 1) * Tc)
                    mul_eng = nc.vector if mul_idx % 16 < 9 else nc.gpsimd
                    mul_idx += 1
                    mul_eng.tensor_mul(ot[:, sl], xt[:, sl], gts[cb][:])
                nc.scalar.dma_start(
                    ov[b, :, :, th * Tc:(th + 1) * Tc], ot3)
    nc.compile()
    return nc


def make_in_maps(x, w1, b1, w2, b2, cw, n_cores=N_CORES):
    """Host-side shard + weight prep. Returns per-core input maps."""
    a = 1.0 / cw
    w1sT = np.ascontiguousarray((w1 * a).T, dtype=np.float32)      # [C, CSQ]
    b1c = np.ascontiguousarray(b1.reshape(-1, 1), dtype=np.float32)
    w2T = np.ascontiguousarray(w2.T, dtype=np.float32)             # [CSQ, C]
    ncb = w2.shape[0] // P
    b2c = np.ascontiguousarray(b2.reshape(ncb, P).T, dtype=np.float32)  # [P, NCB]
    b_loc = x.shape[0] // n_cores
    return [
        {
            "x": np.ascontiguousarray(x[i * b_loc:(i + 1) * b_loc], dtype=np.float32),
            "w1sT": w1sT, "b1": b1c, "w2T": w2T, "b2": b2c,
        }
        for i in range(n_cores)
    ]


_NC_CACHE = {}


def kernel(x, w1, b1, w2, b2, context_window):
    cw = int(context_window)
    x = np.asarray(x)
    key = (cw, x.shape)
    if key not in _NC_CACHE:
        _NC_CACHE[key] = build_nc(x.shape[0] // N_CORES, cw)
    nc = _NC_CACHE[key]
    in_maps = make_in_maps(
        np.asarray(x), np.asarray(w1), np.asarray(b1),
        np.asarray(w2), np.asarray(b2), cw)
    res = run_bass_kernel_spmd(nc, in_maps, core_ids=list(range(N_CORES)))
    return np.concatenate([r["out"] for r in res.results], axis=0)
